# revision 1
# baseline (speedup 1.0000x reference)
"""Trainium2 Bass kernel for nn_MoEModel (conv feature extractor + top-2 MoE).

Strategy (8 NeuronCores):
  - Data-parallel conv trunk: each core runs conv1/pool/conv2/pool on its
    16-image batch shard (fp32, PE-array row/col packing for the small-K
    convolutions).
  - AllGather of flattened features h [128, 12544] in bf16 (the expert
    matmul consumes bf16 anyway; halves collective + reload traffic).
  - Expert-parallel MoE: core e holds expert e's weights [12544, 1000] and
    computes logits for ALL 128 samples (bf16 matmul, fp32 accumulate).
    Gate logits are computed in full fp32 (top-2 selection is numerically
    sensitive).  Each core masks its expert output by its top-2 gate weight
    and a ReduceScatter(+) combines; each core softmaxes its 16-row shard.
"""

import numpy as np

from concourse import bass, bacc, mybir
from concourse.tile import TileContext
from concourse.masks import make_identity
from concourse.bass_utils import run_bass_kernel_spmd

F32 = mybir.dt.float32
F32R = mybir.dt.float32r
BF16 = mybir.dt.bfloat16
AX = mybir.AxisListType
ALU = mybir.AluOpType
ACTF = mybir.ActivationFunctionType

B = 128          # global batch
SH = 16          # batch shard per core
E = 8            # experts == cores
C = 1000         # classes
D = 14 * 14 * 64 # 12544 flattened features
NK = D // 128    # 98 contraction chunks
RG = [list(range(E))]

# W prefetch ring depth (bf16 chunks of [128, 1000])
WBUFS = 48


def _ap(tensor, offset, dims):
    return bass.AP(tensor=tensor, offset=offset, ap=dims)


def _conv_trunk(nc, tc, x16, w1sb, b1sb, w2sb, b2sb, idsb, h_local, h_all,
                do_ag, pump=None):
    """conv1+pool+conv2+pool+transpose for the 16-image shard -> h_local.

    conv1: fp32, 4 images packed via row+col tile_position strips.
    conv2: f32r (4x faster PE), 4-way row-strip packing, per-image psum
    at base partition 0 (f32r matmul requires dst partition base 0).
    """
    with (
        tc.tile_pool(name="conv", bufs=1) as cv,
        tc.tile_pool(name="cps", bufs=2, space="PSUM") as cps,
    ):
        w2r = cv.tile([128, 1152], F32R, tag="w2r", bufs=1)
        nc.vector.tensor_copy(w2r[:], w2sb[:])
        for g in range(4):          # 4 groups of 4 images
            # im strip (dy,dx) = 62 rows x 64 cols (2 junk cols, masked by
            # the x-pool below) so each load is one contiguous 15.9KB run.
            im = cv.tile([36, 3968], F32, tag="im", bufs=2)
            for j in range(4):
                img = 4 * g + j
                for dy in range(3):
                    nc.scalar.dma_start(
                        im[9 * j + 3 * dy:9 * j + 3 * dy + 3, :],
                        _ap(x16, img * 4104 + dy * 64,
                            [[1, 3], [1, 3968]]),
                    )
            c1a = cv.tile([128, 3968], F32, tag="c1a", bufs=1)
            # one matmul per pixel tile: block-diagonal w1 [36, 128] computes
            # all 4 images' 32 channels in a single 36-row contraction
            for t in range(8):      # N tiles over 62x64 pixels
                c0 = t * 496
                ntile = 496
                ps = cps.tile([128, 512], F32, tag="c1ps")
                nc.tensor.matmul(
                    ps[:, 0:ntile],
                    w1sb[:, :],
                    im[:, c0:c0 + ntile],
                    start=True, stop=True,
                )
                nc.scalar.activation(
                    c1a[:, c0:c0 + ntile], ps[:, 0:ntile],
                    ACTF.Relu, bias=b1sb[:], scale=1.0,
                )
            # maxpool 2x2: 62x62 -> 31x31  (fm1 written as f32r for conv2)
            m1 = cv.tile([128, 62 * 31], F32, tag="m1", bufs=1)
            v = c1a[:].rearrange("p (y x) -> p y x", y=62)
            m1v = m1[:].rearrange("p (y x) -> p y x", y=62)
            nc.vector.tensor_max(m1v, v[:, :, 0:62:2], v[:, :, 1:62:2])
            fm1 = cv.tile([128, 961], F32R, tag="fm1", bufs=2)
            m1r = m1[:].rearrange("p (y x) -> p y x", y=62)
            fm1v = fm1[:].rearrange("p (y x) -> p y x", y=31)
            nc.vector.tensor_max(fm1v, m1r[:, 0:62:2, :], m1r[:, 1:62:2, :])

            # ---- conv2 (f32r), image PAIRS via block-diagonal w2 [64, 128]
            # (two images' 32 in-channels contract to their 64 out-channels
            # in one matmul; w2r rows 64b.. hold a copy for pair b) ----
            fm1y = fm1[:].rearrange("p (y x) -> p y x", y=31)
            for bb in range(2):
                c2a = cv.tile([128, 29 * 28], F32, tag="c2a", bufs=2)
                for (r0, nr) in ((0, 17), (17, 12)):
                    ps2 = cps.tile([128, 512], F32, tag=f"c2ps{bb}", bufs=1)
                    for tap in range(9):
                        dy, dx = tap // 3, tap % 3
                        rhs = fm1y[64 * bb:64 * bb + 64,
                                   r0 + dy:r0 + dy + nr,
                                   dx:dx + 28]
                        nc.tensor.matmul(
                            ps2[0:128, 0:nr * 28],
                            w2r[64 * bb:64 * bb + 64,
                                128 * tap:128 * tap + 128],
                            rhs,
                            start=(tap == 0), stop=(tap == 8),
                            tile_position=(64 * bb, 0),
                        )
                    nc.scalar.activation(
                        c2a[:, r0 * 28:(r0 + nr) * 28],
                        ps2[0:128, 0:nr * 28],
                        ACTF.Relu, bias=b2sb[:], scale=1.0,
                    )
                # maxpool 2x2 on 28x28 of the 29x29 grid -> 14x14 (pair-wide)
                m2 = cv.tile([128, 29 * 14], F32, tag="m2", bufs=1)
                cv2v = c2a[:].rearrange("p (y x) -> p y x", y=29)
                m2v = m2[:].rearrange("p (y x) -> p y x", y=29)
                nc.vector.tensor_max(m2v, cv2v[:, :, 0:28:2], cv2v[:, :, 1:28:2])
                fm2 = cv.tile([128, 196], F32, tag="fm2", bufs=2)
                m2r = m2[:].rearrange("p (y x) -> p y x", y=29)
                fm2v = fm2[:].rearrange("p (y x) -> p y x", y=14)
                nc.vector.tensor_max(fm2v, m2r[:, 0:28:2, :], m2r[:, 1:28:2, :])
                for a in range(2):
                    img = 4 * g + 2 * bb + a
                    # transpose [64ch, 196pix] -> h row (pix-major)
                    hst = cv.tile([98, 128], F32, tag="hst", bufs=2)
                    hstb = cv.tile([98, 128], BF16, tag="hstb", bufs=2)
                    for half in range(2):
                        pst = cps.tile([98, 64], F32, tag="pst")
                        nc.tensor.transpose(
                            pst[:],
                            fm2[64 * a:64 * a + 64,
                                98 * half:98 * half + 98],
                            idsb[64 * a:64 * a + 64, 64 * a:64 * a + 64],
                        )
                        nc.vector.tensor_copy(
                            hst[:, 64 * half:64 * half + 64], pst[:]
                        )
                    nc.vector.tensor_copy(hstb[:], hst[:])
                    # off the Pool queue so it never waits behind AllGathers
                    nc.scalar.dma_start(
                        _ap(h_local[g][:].tensor,
                            h_local[g][:].offset + (2 * bb + a) * D,
                            [[64, 98], [98 * 64, 2], [1, 64]]),
                        hstb[:].rearrange("p (h c) -> p h c", h=2),
                    )
            # group g's 4 rows are done on every core: AllGather them now so
            # the wire time hides under the remaining conv groups
            if do_ag:
                nc.gpsimd.collective_compute(
                    "AllGather", ALU.bypass, replica_groups=RG,
                    ins=[h_local[g].opt()], outs=[h_all[g].opt()],
                )
            if pump is not None:
                pump()


def _phase45(nc, tc, do_w, do_rs, wbf_tiles, idsb, gwsb, gbsb, besb, selsb,
             h_all, glog_local, glog_all, cc_in, cc_out, out16):
    # =========== gating (fp32) + expert matmul (bf16) ===========
    with (
        tc.tile_pool(name="hload", bufs=4) as hl,
        tc.tile_pool(name="h32p", bufs=3) as h32p,
        tc.tile_pool(name="hbfp", bufs=3) as hbfp,
        tc.tile_pool(name="gp", bufs=1) as gp,
        tc.tile_pool(name="eps", bufs=1, space="PSUM") as epp,
        tc.tile_pool(name="tps", bufs=2, space="PSUM") as tpp,
    ):
        pse_a = epp.tile([128, 512], F32, tag="pse_a")
        pse_b = epp.tile([128, 488], F32, tag="pse_b")
        if not do_w:
            nc.tensor.matmul(pse_a[:, 0:128], idsb[:], idsb[:],
                             start=True, stop=True)
            nc.tensor.matmul(pse_b[:, 0:128], idsb[:], idsb[:],
                             start=True, stop=True)
        idb = gp.tile([128, 128], BF16, tag="idb")
        nc.vector.tensor_copy(idb[:], idsb[:])
        psg = epp.tile([128, 8], F32, tag="psg")
        hbig = None
        for k in range(NK):
            if k % 4 == 0:
                ncols = min(512, (NK - k) * 128)
                hbig = hl.tile([128, 512], BF16, tag="hbig")
                for g in range(4):
                    nc.scalar.dma_start(
                        hbig[32 * g:32 * g + 32, 0:ncols],
                        h_all[g][:, k * 128:k * 128 + ncols])
            lc = (k % 4) * 128
            pt = tpp.tile([128, 128], BF16, tag="pt")
            nc.tensor.transpose(pt[:], hbig[:, lc:lc + 128], idb[:])
            h32 = h32p.tile([128, 128], F32, tag="h32")
            nc.vector.tensor_copy(h32[:], pt[:])
            hbf = hbfp.tile([128, 128], BF16, tag="hbf")
            nc.vector.tensor_copy(hbf[:], pt[:])
            nc.tensor.matmul(
                psg[:], h32[:], gwsb[:, k * 8:(k + 1) * 8],
                start=(k == 0), stop=(k == NK - 1),
            )
            if do_w:
                wb = wbf_tiles[k]
                nc.tensor.matmul(
                    pse_a[:], hbf[:], wb[:, 0:512],
                    start=(k == 0), stop=(k == NK - 1),
                )
                nc.tensor.matmul(
                    pse_b[:], hbf[:], wb[:, 512:C],
                    start=(k == 0), stop=(k == NK - 1),
                )

        # ---- gate softmax + top-2 mask (all [128, 8] fp32) ----
        g0 = gp.tile([128, 8], F32, tag="g0")
        nc.vector.tensor_add(g0[:], psg[:], gbsb[:])
        gmax = gp.tile([128, 1], F32, tag="gmax")
        nc.vector.reduce_max(gmax[:], g0[:], axis=AX.X)
        gmn = gp.tile([128, 1], F32, tag="gmn")
        nc.vector.tensor_scalar_mul(gmn[:], gmax[:], -1.0)
        gexp = gp.tile([128, 8], F32, tag="gexp")
        gsum = gp.tile([128, 1], F32, tag="gsum")
        nc.scalar.activation(
            gexp[:], g0[:], ACTF.Exp,
            bias=gmn[:], scale=1.0, accum_out=gsum[:],
        )
        grec = gp.tile([128, 1], F32, tag="grec")
        nc.vector.reciprocal(grec[:], gsum[:])
        gg = gp.tile([128, 8], F32, tag="gg")
        nc.vector.tensor_scalar_mul(gg[:], gexp[:], grec[:])
        m1t = gp.tile([128, 1], F32, tag="m1t")
        nc.vector.reduce_max(m1t[:], gg[:], axis=AX.X)
        negsel = gp.tile([128, 8], F32, tag="negsel")
        nc.vector.tensor_scalar(
            negsel[:], gg[:], m1t[:], -2.0,
            op0=ALU.is_equal, op1=ALU.mult,
        )
        masked = gp.tile([128, 8], F32, tag="masked")
        nc.vector.tensor_add(masked[:], gg[:], negsel[:])
        m2t = gp.tile([128, 1], F32, tag="m2t")
        nc.vector.reduce_max(m2t[:], masked[:], axis=AX.X)
        gsel = gp.tile([128, 8], F32, tag="gsel")
        nc.vector.tensor_mul(gsel[:], gg[:], selsb[:])
        ge = gp.tile([128, 1], F32, tag="ge")
        nc.vector.reduce_sum(ge[:], gsel[:], axis=AX.X)
        selm = gp.tile([128, 1], F32, tag="selm")
        nc.vector.tensor_scalar(
            selm[:], ge[:], m2t[:], None, op0=ALU.is_ge,
        )
        wsel = gp.tile([128, 1], F32, tag="wsel")
        nc.vector.tensor_mul(wsel[:], ge[:], selm[:])

        # ---- weighted contribution -> ReduceScatter ----
        contrib = gp.tile([128, C], F32, tag="contrib")
        nc.vector.tensor_add(contrib[:, 0:512], pse_a[:], besb[:, 0:512])
        nc.vector.tensor_add(contrib[:, 512:C], pse_b[:], besb[:, 512:C])
        nc.vector.tensor_scalar_mul(contrib[:], contrib[:], wsel[:])
        nc.gpsimd.dma_start(cc_in[:], contrib[:])
        if do_rs:
            nc.gpsimd.collective_compute(
                "ReduceScatter", ALU.add, replica_groups=RG,
                ins=[cc_in.opt()], outs=[cc_out.opt()],
            )

        # ---- final softmax on the 16-row shard ----
        fin = gp.tile([SH, C], F32, tag="fin")
        nc.gpsimd.dma_start(fin[:], cc_out[:] if do_rs else cc_in[0:SH, :])
        fmax = gp.tile([SH, 1], F32, tag="fmax")
        nc.vector.reduce_max(fmax[:], fin[:], axis=AX.X)
        fmn = gp.tile([SH, 1], F32, tag="fmn")
        nc.vector.tensor_scalar_mul(fmn[:], fmax[:], -1.0)
        fexp = gp.tile([SH, C], F32, tag="fexp")
        fsum = gp.tile([SH, 1], F32, tag="fsum")
        nc.scalar.activation(
            fexp[:], fin[:], ACTF.Exp,
            bias=fmn[:], scale=1.0, accum_out=fsum[:],
        )
        frec = gp.tile([SH, 1], F32, tag="frec")
        nc.vector.reciprocal(frec[:], fsum[:])
        fout = gp.tile([SH, C], F32, tag="fout")
        nc.vector.tensor_scalar_mul(fout[:], fexp[:], frec[:])
        nc.gpsimd.dma_start(out16[:], fout[:])



def build_program(variant="full", repeat=1):
    do_conv = variant not in ("no_conv", "no_conv_no_ag", "expert_only")
    do_ag = variant not in ("no_ag", "no_conv_no_ag", "expert_only",
                            "conv_only", "conv_w")
    if variant == "conv_ag":
        pass
    do_w = variant not in ("no_expert", "conv_only")
    do_rs = variant not in ("no_rs", "expert_only")
    nc = bacc.Bacc("TRN2", target_bir_lowering=False, debug=False, num_devices=E)

    # ---- per-core external I/O ----
    x16 = nc.dram_tensor("x16", [SH, 4104], F32, kind="ExternalInput")
    w1 = nc.dram_tensor("w1", [36, 128], F32, kind="ExternalInput")
    b1 = nc.dram_tensor("b1", [128, 1], F32, kind="ExternalInput")
    w2 = nc.dram_tensor("w2", [128, 1152], F32, kind="ExternalInput")
    b2 = nc.dram_tensor("b2", [128, 1], F32, kind="ExternalInput")
    gw = nc.dram_tensor("gw", [D, 8], F32, kind="ExternalInput")
    gb128 = nc.dram_tensor("gb128", [128, 8], F32, kind="ExternalInput")
    we = nc.dram_tensor("we", [D, C], F32, kind="ExternalInput")
    be128 = nc.dram_tensor("be128", [128, C], F32, kind="ExternalInput")
    sel = nc.dram_tensor("sel", [128, 8], F32, kind="ExternalInput")
    out16 = nc.dram_tensor("out16", [SH, C], F32, kind="ExternalOutput")

    with TileContext(nc) as tc:
        with (
            tc.tile_pool(name="consts", bufs=1) as cp,
            tc.tile_pool(name="wraw", bufs=3) as wraw,
            tc.tile_pool(name="wbf", bufs=WBUFS) as wbf,
            tc.tile_pool(name="dram", bufs=1, space="DRAM") as dp,
        ):
            # ---- constants into SBUF ----
            w1sb = cp.tile([36, 128], F32, tag="w1sb")
            nc.gpsimd.dma_start(w1sb[:, :], w1[:, :])
            w2sb = cp.tile([128, 1152], F32, tag="w2sb")
            nc.gpsimd.dma_start(w2sb[:, :], w2[:, :])
            b1sb = cp.tile([128, 1], F32, tag="b1sb")
            nc.gpsimd.dma_start(b1sb[:], b1[:, :])
            b2sb = cp.tile([128, 1], F32, tag="b2sb")
            nc.gpsimd.dma_start(b2sb[:], b2[:, :])
            idsb = cp.tile([128, 128], F32, tag="idsb")
            make_identity(nc, idsb[:])
            # gate weights laid out [128, 98*8]: col k*8+j = gw[128k+p, j]
            gwsb = cp.tile([128, NK * 8], F32, tag="gwsb")
            nc.gpsimd.dma_start(
                gwsb[:], _ap(gw, 0, [[8, 128], [128 * 8, NK], [1, 8]])
            )
            gbsb = cp.tile([128, 8], F32, tag="gbsb")
            nc.gpsimd.dma_start(gbsb[:], gb128[:, :])
            besb = cp.tile([128, C], F32, tag="besb")
            nc.gpsimd.dma_start(besb[:], be128[:, :])
            selsb = cp.tile([128, 8], F32, tag="selsb")
            nc.gpsimd.dma_start(selsb[:], sel[:, :])

            # ---- DRAM bounce buffers for collectives ----
            h_local = [dp.tile([4, D], BF16, name=f"h_local{g}",
                               tag=f"h_local{g}") for g in range(4)]
            glog_local = dp.tile([SH, 8], F32, tag="glog_local")
            cc_in = dp.tile([B, C], F32, tag="cc_in")
            cc_out = dp.tile([SH, C], F32, tag="cc_out")

            for _rep in range(repeat):
                h_all = [dp.tile([32, D], BF16,
                                 name=f"h_all{_rep}_{g}",
                                 tag=f"h_all{_rep}_{g}", addr_space="Shared")
                         for g in range(4)]
                # ---- expert weight stream: fp32 chunk DMA -> bf16 ring.
                # Emitted interleaved with the conv groups (pump) so the DVE
                # casts don't starve the conv pool ops, while the SP queue
                # still prefetches W throughout the conv phase. ----
                wbf_tiles = []
                _pumped = [0]

                def pump(n=12, _rep=_rep):
                    if not do_w:
                        return
                    # paired chunk loads: one [128, 2000] DMA covers two
                    # contraction chunks, halving per-DMA fixed overhead
                    hi = min(NK, _pumped[0] + n)
                    k = _pumped[0]
                    while k < hi:
                        wt2 = wraw.tile([128, 2000], F32, tag="wt", bufs=2)
                        nc.sync.dma_start(
                            wt2[:].rearrange("p (t c) -> p t c", t=2),
                            _ap(we, k * 128 * C,
                                [[C, 128], [128 * C, 2], [1, C]]),
                        )
                        for t in range(2):
                            wb = wbf.tile([128, C], BF16, tag="wb")
                            nc.vector.tensor_copy(
                                wb[:], wt2[:, C * t:C * t + C])
                            wbf_tiles.append(wb)
                        k += 2
                    _pumped[0] = k

                # ==== conv trunk (per-group AllGathers emitted inside) ====
                if do_conv:
                    pump(8)
                    _conv_trunk(nc, tc, x16, w1sb, b1sb, w2sb, b2sb, idsb,
                                h_local, h_all, do_ag, pump=lambda: pump(12))
                pump(NK)
                glog_all = dp.tile([B, 8], F32, tag=f"glog_all{_rep}",
                                   addr_space="Shared")

                if variant not in ("conv_only", "conv_ag", "conv_w"):
                    _phase45(nc, tc, do_w, do_rs, wbf_tiles, idsb, gwsb,
                             gbsb, besb, selsb, h_all,
                             glog_local, glog_all, cc_in, cc_out, out16)

    nc.compile()
    return nc


_NC_CACHE = None


def _get_program():
    global _NC_CACHE
    if _NC_CACHE is None:
        _NC_CACHE = build_program()
    return _NC_CACHE


def make_in_maps(x, conv1_w, conv1_b, conv2_w, conv2_b,
                 gate_w, gate_b, expert_w, expert_b):
    x = np.asarray(x, np.float32).reshape(B, 4096)
    # 8 zero floats of row padding so conv1's 62x64 strips stay in-bounds
    x = np.concatenate([x, np.zeros((B, 8), np.float32)], axis=1)
    w1_9x32 = np.asarray(conv1_w, np.float32).reshape(9, 32)
    w1 = np.zeros((36, 128), np.float32)
    for j in range(4):
        w1[9 * j:9 * j + 9, 32 * j:32 * j + 32] = w1_9x32
    b1 = np.ascontiguousarray(
        np.tile(np.asarray(conv1_b, np.float32), 4).reshape(128, 1))
    w2r9 = np.asarray(conv2_w, np.float32).reshape(9, 32, 64)
    w2 = np.zeros((128, 1152), np.float32)
    for bpair in (0, 1):
        for a in (0, 1):
            blk = w2[64 * bpair + 32 * a:64 * bpair + 32 * a + 32]
            blk = blk.reshape(32, 9, 128)
            blk[:, :, 64 * a:64 * a + 64] = w2r9.transpose(1, 0, 2)
    b2 = np.ascontiguousarray(
        np.tile(np.asarray(conv2_b, np.float32), 2).reshape(128, 1))
    gw = np.ascontiguousarray(np.asarray(gate_w, np.float32))
    gb128 = np.ascontiguousarray(
        np.broadcast_to(np.asarray(gate_b, np.float32), (128, 8)))
    ew = np.asarray(expert_w, np.float32)
    eb = np.asarray(expert_b, np.float32)
    in_maps = []
    for r in range(E):
        onehot = np.zeros((1, 8), np.float32)
        onehot[0, r] = 1.0
        in_maps.append({
            "x16": np.ascontiguousarray(x[r * SH:(r + 1) * SH]),
            "w1": w1, "b1": b1, "w2": w2, "b2": b2,
            "gw": gw, "gb128": gb128,
            "we": np.ascontiguousarray(ew[r]),
            "be128": np.ascontiguousarray(
                np.broadcast_to(eb[r], (128, C))),
            "sel": np.ascontiguousarray(np.broadcast_to(onehot, (128, 8))),
        })
    return in_maps


def assemble_out(outs):
    """Concat per-core out16 shards (sample order s = 32g + 4r + j from the
    per-group AllGather layout) and restore original batch order."""
    allo = np.concatenate([np.asarray(o) for o in outs], axis=0)
    s = np.arange(B)
    orig = 16 * ((s % 32) // 4) + 4 * (s // 32) + (s % 4)
    res = np.empty_like(allo)
    res[orig] = allo
    return res


def kernel(**inputs):
    nc = _get_program()
    in_maps = make_in_maps(**inputs)
    res = run_bass_kernel_spmd(nc, in_maps, core_ids=list(range(E)))
    return assemble_out([res.results[r]["out16"] for r in range(E)])

